# revision 46
# baseline (speedup 1.0000x reference)
"""GAT-style dense attention kernel for TRN2 (8 NeuronCores, SPMD over batch).

Reference computation (B=N=256, F=128, H=4, D=8):
  q = x@Wq+bq; k = x@Wk+bk; v = x@Wv+bv          (per-head dim D=8)
  s = einsum('bqhd,bkhd->bhqk', q, k)/sqrt(D)
  s = where(adj[q,k]==0, -inf, s)                 (adj shared across b,h)
  a = softmax(s, -1)
  out = einsum('bhqk,bkhd->bqhd', a, v).reshape(B,N,H*D) @ Wo + bo

Kernel strategy v2 (per core: 32 batches, software-pipelined, work spread
over all four compute engines; steady state ~1.87us/batch):
  - host: xT = x.transpose -> [b, F, N]; batch 0's xT block rides the same
    (first) hot DMA as the projection weights so the pipeline starts early
  - extended q~/k~ projections, spread layout [128, 2, 256], head h on
    partitions 32h..32h+9: q~ row 8 = scale*(xWq+bq).bk_h (extra weight
    column + bias), k~ row 8 = 1.0 (bias) => q~ . k~ == scaled biased
    scores with the whole bias algebra folded into ONE PSUM->SBUF
    tensor_add (DVE, per-partition bias vector)
  - additive -20 adjacency mask written into every score quarter by fp8
    DoubleRow identity matmuls at 0.208 ns/col (out[m,n] = sum_k,t
    lhs[k,t,m]*rhs[k,t,n]; lhs[:,0,:]=I selects rhs[:,0,:]=maskT); the
    fp8 operands ship as a NATIVE fp8 dram param (bitcast views of f32
    tiles read as zeros in the ldweights path)
  - 9-row f32r score matmuls accumulate onto the mask, 4 heads packed at
    PE tile rows 32h
  - exp split by head pair: ACT does exact exp for pair 0 (one [128,1024]
    inst) and pair-1 hh0; DVE computes pair-1 hh1 via the Schraudolph
    bit trick bits(bf16 e^s) = round(s*2^7/ln2 + B16) as one
    tensor_scalar (mult+add -> int16). pair-1 scores live in TWO PSUM
    tiles and land in TWO SBUF E tiles: Tile serializes cross-engine
    readers/writers of one PSUM/SBUF tile, which would stall the pipe.
    The hh1 (Schraudolph) quarters are written first each iteration so
    the single-buffer S1b ring never gates the next sexp.
  - V and Wo fused on host (Wvo_h = Wv_h @ Wo_h); ones column yields the
    softmax row-sums; bv@Wo + bo/H ride the vw bias (bo/H divides out
    through the normalization, killing the +bo pass)
  - P9 matmuls with E stationary ([128k x 128q] bf16) and the 9-column
    V-block moving -> natural [q, (c2, h, 1+D)] layout in a PSUM bank
    shared with the V*Wo matmuls
  - normalize: one strided DVE ppcopy + one reciprocal cover TWO batches
    of the paired vp bank (the pp regions sit 128 cols apart), halving
    the per-instruction PSUM access-latency charge; Pool engine does the
    per-head scale multiply and head-sum add tree straight into the
    output staging tile (Pool has no PSUM port, so it only touches
    SBUF; the drain-critical last two batches normalize on DVE);
    DMA out every 4 batches
  - PSUM banks (8 total): pair-0 scores ring-2 (4), pair-1 hh0/hh1
    rings-1 (2), qk projections (1), vw+pp shared bank (1)
"""

import sys

sys.path.insert(0, "/opt/trn_rl_repo")

import os

import numpy as np

import concourse.bass as bass
import concourse.tile as tile
from concourse import mybir
from concourse.bass_utils import run_bass_kernel_spmd
from concourse.tile_rust import add_dep_helper


def _dep(from_inst, to_inst, reason):
    if from_inst is None or to_inst is None:
        return
    add_dep_helper(
        getattr(from_inst, "ins", from_inst),
        getattr(to_inst, "ins", to_inst),
        sync=False,
        reason=reason,
    )

DEBUG_LABELS = {}


def _lbl(inst, label):
    if inst is not None:
        m = getattr(inst, "ins", inst)
        DEBUG_LABELS[getattr(m, "name", "?")] = label
    return inst


B = 256
N = 256
F = 128
H = 4
D = 8
NCORES = 8
BPC = B // NCORES  # batches per core
MASK_NEG = -20.0
# Schraudolph bf16-exp constants: bits(bf16 e^s) ~= round(s * 2^7/ln2 + B16)
SCH_A16 = 184.6618
SCH_B16 = 16250.5
# cols of pair-1 scores (of 1024) exp'd exactly on ACT; rest Schraudolph on DVE
EXP_ACT = 512
WARM_REPS = 4

f32 = mybir.dt.float32
f32r = mybir.dt.float32r
bf16 = mybir.dt.bfloat16
fp8e4 = mybir.dt.float8e4
i16 = mybir.dt.int16

# cblob section offsets (f32 columns); fp8/bf16 consts ship as their own
# natively-typed dram params (matmul operands via bitcast views of an f32
# tile read as zeros on hardware)
OFF_WQS = 0          # [128, 128] f32r ext q~ spread weights (hot)
OFF_WKS = 128        # [128, 128] f32r ext k~ spread weights (hot)
OFF_BQK = 256        # [128, 2] f32 qk-move bias (hot)
OFF_XT0 = 258        # [128, 256] f32r batch-0 xt block (hot)
HOT_COLS = 514
OFF_WVO = 514        # [128, 36] f32r fused V*Wo weights
OFF_BVOF = 550       # [128, 72] f32 vw bias (bv@Wo + bo/H + ones col)
TOT_COLS = 622

# the quarter (p, hh, c) masked post-exp on Pool instead of PE-additive
# (None: all quarters PE-additive -- the Pool multiply adds a serial
# sexp->pmask->pp->ppcopy hop that outweighs the 53ns PE saving)
POOL_Q = None


def _build_consts(edge_index, Wq, bq, Wk, bk, Wv, bv, Wo, bo):
    import ml_dtypes

    scale = 1.0 / np.sqrt(np.float32(D))

    # extended spread projection weights; head h at partitions 32h..32h+9
    Wq_s = np.zeros((F, 128), np.float32)
    Wk_s = np.zeros((F, 128), np.float32)
    bqk = np.zeros((128, 2), np.float32)
    for h in range(H):
        wq_h = Wq[:, 8 * h : 8 * h + 8]  # [F, 8]
        bq_h = bq[8 * h : 8 * h + 8]
        bk_h = bk[8 * h : 8 * h + 8]
        for d in range(D):
            Wq_s[:, 32 * h + d] = wq_h[:, d] * scale
            Wk_s[:, 32 * h + d] = Wk[:, 8 * h + d]
            bqk[32 * h + d, 0] = bq_h[d] * scale
        # q~ row 8: (scale*(xWq+bq)) . bk_h  (linear part + bias)
        Wq_s[:, 32 * h + 8] = scale * (wq_h @ bk_h)
        bqk[32 * h + 8, 0] = scale * float(bq_h @ bk_h)
        # k~ row 8: constant 1.0 via the move bias
        bqk[32 * h + 8, 1] = 1.0

    # fused V*Wo, 9 columns per head: col 9h+0 reserved (ones), 9h+1+j = VWo
    # bv@Wo and bo/H ride the bias; ones col yields softmax row-sums
    Wvo = np.zeros((F, 9 * H), np.float32)
    bvo = np.zeros((1, 9 * H), np.float32)
    for h in range(H):
        wv_h = Wv[:, 8 * h : 8 * h + 8]  # [F, 8]
        wo_h = Wo[8 * h : 8 * h + 8, :]  # [8, 8]
        Wvo[:, 9 * h + 1 : 9 * h + 9] = wv_h @ wo_h
        bvo[0, 9 * h + 1 : 9 * h + 9] = bv[8 * h : 8 * h + 8] @ wo_h + bo / H
        bvo[0, 9 * h + 0] = 1.0  # ones column -> softmax row-sums
    bvo_full = np.broadcast_to(np.tile(bvo, (1, 2)), (128, 2 * 9 * H)).copy()

    # adjacency; additive mask M^T[k, q] packed as [128, c, 256] (kchunk, q)
    adj = np.zeros((B, B), np.float32)
    adj[edge_index[0], edge_index[1]] = 1.0
    maskT = np.where(adj.T == 0.0, np.float32(MASK_NEG), np.float32(0.0))  # [k, q]
    maskT_p = np.ascontiguousarray(maskT.reshape(2, 128, 256).transpose(1, 0, 2))

    # DoubleRow fp8 operands: lhs [128, 2, 128] = (I, 0); rhs [128, 2, 2, 256]
    # = per-c (maskT_p[:, c, :], 0)
    drl = np.zeros((128, 2, 128), np.float32)
    drl[:, 0, :] = np.eye(128)
    drr = np.zeros((128, 2, 2, 256), np.float32)
    for c in range(2):
        drr[:, c, 0, :] = maskT_p[:, c, :]
    drl8 = drl.astype(ml_dtypes.float8_e4m3).reshape(128, 256)
    drr8 = drr.astype(ml_dtypes.float8_e4m3).reshape(128, 1024)

    # binary adjacency (bf16) for the Pool-masked quarter (c chunk 1)
    adjq = (maskT_p[:, 1, :] == 0.0).astype(ml_dtypes.bfloat16)  # [128,256]

    def as_f32(a):  # bit-pack raw bytes into f32 columns
        b = np.ascontiguousarray(a).view(np.uint8).reshape(128, -1)
        return b.view(np.float32)

    cblob = np.concatenate(
        [Wq_s, Wk_s, bqk, np.zeros((128, 256), np.float32), Wvo, bvo_full],
        axis=1,
    ).astype(np.float32)
    assert cblob.shape == (128, TOT_COLS), cblob.shape
    dr8 = np.concatenate([drl8, drr8], axis=1)  # [128, 1280] fp8
    # xt block 0 is appended per-core in kernel() so the first projection's
    # input rides the same (first) DMA as the hot weights
    return dict(
        cblob=np.ascontiguousarray(cblob),
        dr8=np.ascontiguousarray(dr8),
    )


_ENG_SEM = {
    "PE_44": mybir.EngineType.PE,
    "DVE_44": mybir.EngineType.DVE,
    "Activation_44": mybir.EngineType.Activation,
    "Pool_44": mybir.EngineType.Pool,
    "SP_44": mybir.EngineType.SP,
}


def _prune_stale_self_waits(nc, margin=6):
    """Drop waits on an instruction's OWN engine semaphore when the waited-on
    completion is at least `margin` same-engine instructions in the past:
    engines execute in order, so the in-order pipeline (and its write-ack
    drain) already guarantees those. Cuts the number of injected wait-carrier
    NoOps (the ISA has a single wait slot per instruction)."""
    f = nc.m.functions[0]
    for bb in f.blocks:
        counts = {}
        for inst in bb.instructions:
            eng = inst.engine
            counts[eng] = counts.get(eng, 0) + 1
            si = getattr(inst, "sync_info", None)
            if si is None or not si.on_wait:
                continue
            keep = [
                w
                for w in si.on_wait
                if not (
                    _ENG_SEM.get(w.ant_name) == eng
                    and counts[eng] - w.wait_value >= margin
                )
            ]
            if len(keep) != len(si.on_wait):
                inst.sync_info = mybir.SyncInfo(
                    on_wait=keep, on_update=si.on_update
                )


def _split_excess_waits(nc, max_waits=1):
    """Walrus allows only 2 sync-wait slots per engine instruction. Tile's
    vector-clock wait emission occasionally exceeds that (schedule-dependent);
    hoist the excess onto injected same-engine NoOps placed just before."""
    _prune_stale_self_waits(nc)
    f = nc.m.functions[0]
    for bb in f.blocks:
        insts = list(bb.instructions)
        n_inserted = 0
        for idx, inst in enumerate(insts):
            si = getattr(inst, "sync_info", None)
            if si is None or not si.on_wait or len(si.on_wait) <= max_waits:
                continue
            waits = list(si.on_wait)
            keep, excess = waits[:max_waits], waits[max_waits:]
            pos = idx + n_inserted
            while excess:
                chunk, excess = excess[:max_waits], excess[max_waits:]
                nop = mybir.InstNoOp(
                    name=nc.get_next_instruction_name(),
                    ins=[],
                    outs=[],
                    engine=inst.engine,
                    sync_info=mybir.SyncInfo(on_wait=chunk, on_update=[]),
                    bass_nofuse=True,
                )
                bb.instructions.insert(pos, nop)
                pos += 1
                n_inserted += 1
            inst.sync_info = mybir.SyncInfo(on_wait=keep, on_update=si.on_update)


def _build_program():
    nc = bass.Bass()
    global NORM_ENG, USE_SCHR, USE_DR
    import os
    NORM_ENG = nc.vector if os.environ.get("NORM_DVE", "0") == "1" else nc.gpsimd
    USE_SCHR = os.environ.get("SCHR", "1") == "1"
    USE_DR = os.environ.get("DR", "1") == "1"

    x_t = nc.declare_dram_parameter("xt", [BPC, F, N], f32r, isOutput=False)
    out = nc.declare_dram_parameter("out", [BPC, N, D], f32, isOutput=True)
    DEBUG = os.environ.get("KDEBUG", "0") == "1"
    if DEBUG:
        dbg_qk = nc.declare_dram_parameter("dbg_qk", [128, 2, 256], f32, isOutput=True)
        dbg_e0 = nc.declare_dram_parameter("dbg_e0", [128, 2, 2, 256], f32, isOutput=True)
        dbg_e1a = nc.declare_dram_parameter("dbg_e1a", [128, 2, 256], f32, isOutput=True)
        dbg_e1b = nc.declare_dram_parameter("dbg_e1b", [128, 2, 256], f32, isOutput=True)
        dbg_ppc = nc.declare_dram_parameter("dbg_ppc", [128, 2, H, 9], f32, isOutput=True)
    c_blob = nc.declare_dram_parameter(
        "cblob", [128, TOT_COLS], f32r, isOutput=False
    )
    d_dr8 = nc.declare_dram_parameter("dr8", [128, 1280], fp8e4, isOutput=False)

    with tile.TileContext(nc) as tc:
        with (
            tc.tile_pool(name="consts", bufs=1) as cpool,
            tc.tile_pool(name="xt", bufs=33) as xt_pool,
            tc.tile_pool(name="qk", bufs=16) as qk_pool,
            tc.tile_pool(name="vw", bufs=12) as vw_pool,
            tc.tile_pool(name="E", bufs=12) as e_pool,
            tc.tile_pool(name="Eh", bufs=24) as eh_pool,
            tc.tile_pool(name="small", bufs=32) as sm_pool,
            tc.tile_pool(name="ostage", bufs=4) as ost_pool,
            tc.tile_pool(name="ps_qk", bufs=1, space="PSUM") as ps_qk_pool,
            tc.tile_pool(name="ps_vp", bufs=1, space="PSUM") as ps_vp_pool,
            tc.tile_pool(name="ps_s", bufs=3, space="PSUM") as ps_s,
        ):
            # hot consts (projection weights) first on the SP queue; first xt
            # block on the ACT DGE queue; cold consts follow on SP
            cblob = cpool.tile([128, TOT_COLS], f32r, tag="cblob")
            # hot DMA carries the projection weights AND batch-0's xt block
            nc.sync.dma_start(
                out=cblob[:, 0:HOT_COLS], in_=c_blob[:, 0:HOT_COLS]
            )
            xt0 = cblob[:, OFF_XT0 : OFF_XT0 + 256].rearrange(
                "p (c n) -> p c n", c=2
            )
            dr_t = cpool.tile([128, 1280], fp8e4, tag="dr_t")
            nc.sync.dma_start(out=dr_t[:], in_=d_dr8[:])

            wqs = cblob[:, OFF_WQS : OFF_WQS + 128]
            wks = cblob[:, OFF_WKS : OFF_WKS + 128]
            bqk = cblob[:, OFF_BQK : OFF_BQK + 2].bitcast(f32)
            wvo = cblob[:, OFF_WVO : OFF_WVO + 36]
            bvof = cblob[:, OFF_BVOF : OFF_BVOF + 72].bitcast(f32)
            drl = dr_t[:, 0:256].rearrange("p (t m) -> p t m", t=2)
            drr = dr_t[:, 256:1280].rearrange("p (c t n) -> p c t n", c=2, t=2)
            adjq = None

            # PE p-state warmup: tiny matmuls as early as possible start the
            # tensor-engine clock ramp; warm data memset on the idle Pool
            # engine so it does not wait on anything
            if WARM_REPS:
                warm_ps = ps_vp_pool.tile([128, 512], f32, tag="vp")
                warm_sb = cpool.tile([8, 512], f32r, tag="warm_sb")
                nc.gpsimd.memset(warm_sb.bitcast(f32)[:], 0.0)
                for w in range(WARM_REPS):
                    nc.tensor.matmul(
                        warm_ps[0:8, :], warm_sb[:, 0:8], warm_sb[:],
                        start=True, stop=True, skip_group_check=(w > 0),
                    )

            # Make DVE and ACT observe the const-DMA queue once, so the
            # const-load ticks drop out of every later wait list (Tile's
            # vector-clock waits are not transitive across engines).
            obs = cpool.tile([1, 8], f32, tag="obs")
            nc.vector.tensor_copy(obs[:, 0:2], cblob[0:1, 0:2].bitcast(f32))
            nc.scalar.copy(obs[:, 4:6], cblob[0:1, 2:4].bitcast(f32))

            # per-batch state, filled by the stage emitters below
            xt_sb = {}      # b -> xt tile
            qk_ps = {}      # b -> PSUM qk tile
            vp_ps = {}      # b -> PSUM vw tile (vw [0:72])
            qk_sb = {}      # b -> SBUF qk tile
            vw_sb = {}      # b -> SBUF vw tile
            s_ps = {}       # (b, p) -> PSUM scores tile
            e_sb = {}       # (b, p) -> SBUF exp tile
            ppc_of = {}     # b -> SBUF pp copy tile
            rec_of = {}     # b -> SBUF reciprocal tile
            st = {"ostage": None, "ppcopy": None, "obsb": None}
            sc_last = {}

            def emit_xt(b):
                if not 0 <= b < BPC or b in xt_sb:
                    return
                t = xt_pool.tile([128, 2, 128], f32r, tag="xt")
                nc.sync.dma_start(
                    out=t[:], in_=x_t[b].rearrange("f (c n) -> f c n", c=2)
                )
                xt_sb[b] = t

            def emit_qkmm(b):
                # extended q~/k~ spread projections into one PSUM bank
                if not 0 <= b < BPC:
                    return
                t = ps_qk_pool.tile([128, 2, 256], f32, tag="qkp")
                xt_flat = xt_sb[b].rearrange("f c n -> f (c n)")
                _lbl(nc.tensor.matmul(t[:, 0, :], wqs, xt_flat, start=True, stop=True), f"qmm({b})")
                _lbl(nc.tensor.matmul(
                    t[:, 1, :], wks, xt_flat,
                    start=True, stop=True, skip_group_check=True,
                ), f"kmm({b})")
                qk_ps[b] = t

            def emit_qkmove(b):
                # PSUM -> SBUF with the per-partition q~/k~ bias (incl the
                # k~ ones row) added in the move
                if not 0 <= b < BPC:
                    return
                t = qk_pool.tile([128, 2, 256], f32r, tag="qk")
                i = _lbl(nc.vector.tensor_add(
                    t[:], qk_ps[b][:],
                    bqk.unsqueeze(2).to_broadcast([128, 2, 256]),
                ), f"qkmove({b})")
                pass
                if st["obsb"] is None:
                    # absorb the xt DMA queue tick on DVE once
                    ob = sm_pool.tile([1, 2], f32, tag="obsb")
                    iob = nc.vector.tensor_copy(
                        ob[:], xt_sb[2][0:1, 0, 0:2].bitcast(f32)
                    )
                    _dep(i, iob, "absorb xt DMASW tick on DVE")
                    st["obsb"] = iob
                qk_sb[b] = t

            def emit_vwmm(b):
                # fused V*Wo projection into the shared vw+pp PSUM bank
                if not 0 <= b < BPC or b in vp_ps:
                    return
                if b % 2 == 0:
                    t = ps_vp_pool.tile([128, 512], f32, tag="vp")
                else:
                    t = vp_ps[b - 1]
                base = 128 * (b % 2)
                for c in range(2):
                    i_vw = _lbl(nc.tensor.matmul(
                        t[:, base + 36 * c : base + 36 * c + 36],
                        xt_sb[b][:, c, :], wvo,
                        start=True, stop=True, skip_group_check=True,
                    ), f"vwmm({b},{c})")
                    # don't let this slip between a mask and its scores on
                    # PE: the pair-1 score stream feeds the pacing exp
                    _dep(i_vw, sc_last.get((b, 1)), "vwmm after own scores")
                vp_ps[b] = t

            def emit_vwmove(b):
                if not 0 <= b < BPC or b in vw_sb:
                    return
                base = 128 * (b % 2)
                t = vw_pool.tile([128, 2, 9 * H], bf16, tag="vw")
                _lbl(nc.vector.tensor_add(
                    t[:],
                    vp_ps[b][:, base : base + 72].rearrange(
                        "p (c v) -> p c v", c=2
                    ),
                    bvof.rearrange("p (c v) -> p c v", c=2),
                ), f"vwmove({b})")
                vw_sb[b] = t

            def emit_scores(b, p):
                # per (hh, c) quarter: fp8 DoubleRow mask write opens the
                # accumulation, the 9-row f32r score matmul closes it; the
                # POOL_Q quarter skips the mask (multiplied post-exp on Pool)
                if not 0 <= b < BPC:
                    return
                t = ps_s.tile([128, 2, 2, 256], f32, tag="S")  # (hh, c, q)
                first = True
                for hh in range(2):
                    h = 2 * p + hh
                    for c in range(2):
                        pooled = (p, hh, c) == POOL_Q
                        if not pooled:
                            _lbl(nc.tensor.matmul(
                                t[:, hh, c, :],
                                drl, drr[:, c],
                                start=True, stop=False,
                                perf_mode=mybir.MatmulPerfMode.DoubleRow,
                                skip_group_check=not first,
                            ), f"mask({b},{p},{hh},{c})")
                            first = False
                        i_sc = _lbl(nc.tensor.matmul(
                            t[:, hh, c, :],
                            qk_sb[b][32 * h : 32 * h + 9, 1, 128 * c : 128 * c + 128],
                            qk_sb[b][32 * h : 32 * h + 9, 0, :],
                            start=pooled, stop=True,
                            skip_group_check=not (pooled and first),
                            tile_position=(32 * h, 0),
                        ), f"sc({b},{p},h{h},c{c})")
                        if pooled and first:
                            first = False
                sc_last[(b, p)] = i_sc
                s_ps[(b, p)] = t

            def emit_exp(b, p):
                # pair 0: one tile, single ACT writer. pair 1: hh0 and hh1 in
                # SEPARATE tiles so the ACT exp and the DVE Schraudolph do not
                # serialize on a shared-tile write-after-write dep.
                if not 0 <= b < BPC:
                    return
                sp = s_ps[(b, p)]
                if p == 0:
                    t = e_pool.tile([128, 2, 2, 256], bf16, tag="E")
                    _lbl(nc.scalar.activation(
                        t[:], sp[:], mybir.ActivationFunctionType.Exp
                    ), f"exp({b},{p})")
                    e_sb[(b, p)] = t
                else:
                    ta = eh_pool.tile([128, 2, 256], bf16, tag="Ea")
                    tb = eh_pool.tile([128, 2, 256], bf16, tag="Eb")
                    _lbl(nc.scalar.activation(
                        ta[:], sp[:, 0], mybir.ActivationFunctionType.Exp,
                    ), f"exp({b},{p})")
                    _lbl(nc.vector.tensor_scalar(
                        tb.bitcast(i16)[:],
                        sp[:, 1],
                        SCH_A16, SCH_B16,
                        op0=mybir.AluOpType.mult, op1=mybir.AluOpType.add,
                    ), f"sexp({b},{p})")
                    e_sb[(b, p)] = (ta, tb)

            def emit_pool_mask(b):
                # binary-adjacency multiply for the unmasked quarter, on Pool
                if POOL_Q is None or not 0 <= b < BPC:
                    return
                p, hh, c = POOL_Q
                t = e_sb[(b, p)][hh]
                _lbl(nc.gpsimd.tensor_mul(
                    t[:, c, :], t[:, c, :], adjq,
                ), f"pmask({b})")

            def emit_pp(b, p):
                # P9': E stationary, V-block moving -> natural [q, (c2,h,9)]
                # into this batch's pp region of the paired vw+pp bank
                if not 0 <= b < BPC:
                    return
                base = 256 + 128 * (b % 2)
                pp = vp_ps[b][:, base : base + 72].rearrange(
                    "p (c2 h j) -> p c2 h j", c2=2, h=H
                )
                e_p = e_sb[(b, p)]
                for hh in range(2):
                    h = 2 * p + hh
                    e_h = e_p[:, hh] if p == 0 else e_p[hh][:]
                    for c2 in range(2):
                        for c in range(2):
                            _lbl(nc.tensor.matmul(
                                pp[:, c2, h, :],
                                e_h[:, c, 128 * c2 : 128 * c2 + 128],
                                vw_sb[b][:, c, 9 * h : 9 * h + 9],
                                start=(c == 0), stop=(c == 1),
                                skip_group_check=True,
                            ), f"pp({b},{p},h{h},c2{c2},c{c})")

            def emit_ppcopy(b):
                if not 0 <= b < BPC:
                    return
                base = 256 + 128 * (b % 2)
                ppv = vp_ps[b][:, base : base + 72].rearrange(
                    "p (c2 h j) -> p c2 h j", c2=2, h=H
                )
                # tiny PSUM->SBUF copy is the only post-pp reader of the
                # paired bank, keeping its release chain short
                ppc = sm_pool.tile([128, 2, H, 9], f32, tag="ppc")
                i = _lbl(nc.vector.tensor_copy(ppc[:], ppv[:]), f"ppcopy({b})")
                st["ppcopy"] = i
                ppc_of[b] = ppc

            def emit_rec(b):
                if not 0 <= b < BPC:
                    return
                ppc = ppc_of[b]
                rec = sm_pool.tile([128, 2, H], f32, tag="rec")
                _lbl(nc.vector.reciprocal(rec[:], ppc[:, :, :, 0]), f"rec({b})")
                rec_of[b] = rec

            def emit_mulred(b):
                # per-head scale + head-sum on the Pool engine, straight into
                # the output staging tile (bo already folded into vw bias)
                if not 0 <= b < BPC:
                    return
                ppc = ppc_of.pop(b)
                rec = rec_of.pop(b)
                eng = NORM_ENG
                tmp = sm_pool.tile([128, 2, D, H], f32, tag="tmp")
                _lbl(eng.tensor_mul(
                    tmp[:],
                    ppc[:, :, :, 1:9].transpose([0, 1, 3, 2]),
                    rec[:].unsqueeze(2).to_broadcast([128, 2, D, H]),
                ), f"mul({b})")
                if b % 4 == 0:
                    ostage = ost_pool.tile([128, 4, 2, D], f32, tag="ost")
                    st["ostage"] = ostage
                # head-sum as a 2-step add tree (Pool tensor_reduce cannot
                # reduce along free axes)
                t2 = sm_pool.tile([128, 2, D, 2], f32, tag="t2")
                _lbl(eng.tensor_add(
                    t2[:], tmp[:, :, :, 0:2], tmp[:, :, :, 2:4]
                ), f"red2({b})")
                _lbl(eng.tensor_add(
                    st["ostage"][:, b % 4, :, :], t2[:, :, :, 0], t2[:, :, :, 1]
                ), f"red({b})")
                if b % 4 == 3:
                    nc.sync.dma_start(
                        out=out[b - 3 : b + 1].rearrange(
                            "b (c p) j -> p b c j", c=2
                        ),
                        in_=st["ostage"][:],
                    )

            def drop(b):
                # release python refs so tile pools can recycle cleanly
                for dd in (xt_sb, qk_ps, qk_sb, vw_sb):
                    dd.pop(b, None)
                if b % 2 == 1:
                    vp_ps.pop(b - 1, None)
                    vp_ps.pop(b, None)
                for p in range(2):
                    s_ps.pop((b, p), None)
                    e_sb.pop((b, p), None)

            # ---- software-pipelined schedule ----
            # prologue: xt blocks ahead of the cold consts in HWDGE order
            xt_sb[0] = xt0
            emit_xt(0)
            emit_xt(1)
            emit_xt(2)
            nc.sync.dma_start(
                out=cblob[:, HOT_COLS:TOT_COLS], in_=c_blob[:, HOT_COLS:TOT_COLS]
            )
            emit_qkmm(0)
            emit_qkmove(0)
            emit_vwmm(0)
            emit_vwmove(0)
            emit_qkmm(1)
            emit_qkmove(1)
            emit_scores(0, 0)
            emit_exp(0, 0)
            emit_scores(0, 1)
            emit_scores(1, 0)
            emit_qkmm(2)
            emit_qkmove(2)
            emit_vwmm(1)
            emit_vwmove(1)

            # single-buffer score rings: each batch's score tiles are
            # written late in the iteration, after the previous batch's
            # exp/schr readers have retired
            for b in range(BPC):
                emit_xt(b + 3)
                emit_exp(b, 1)
                emit_pool_mask(b)
                emit_pp(b, 0)
                emit_qkmm(b + 3)
                emit_vwmm(b + 1)
                emit_vwmove(b + 1)
                emit_pp(b, 1)
                emit_scores(b + 1, 1)
                emit_scores(b + 2, 0)
                emit_exp(b + 1, 0)
                emit_ppcopy(b)
                emit_qkmove(b + 3)
                emit_mulred(b - 2)
                if DEBUG and b == 0:
                    nc.sync.dma_start(out=dbg_qk[:], in_=qk_sb[0][:].bitcast(f32))
                    et = e_sb[(0, 0)]
                    d0 = sm_pool.tile([128, 2, 2, 256], f32, tag="d0")
                    nc.vector.tensor_copy(d0[:], et[:])
                    nc.sync.dma_start(out=dbg_e0[:], in_=d0[:])
                    ta0, tb0 = e_sb[(0, 1)]
                    d1 = sm_pool.tile([128, 2, 256], f32, tag="d1")
                    nc.vector.tensor_copy(d1[:], ta0[:])
                    nc.sync.dma_start(out=dbg_e1a[:], in_=d1[:])
                    d2 = sm_pool.tile([128, 2, 256], f32, tag="d2")
                    nc.vector.tensor_copy(d2[:], tb0[:])
                    nc.sync.dma_start(out=dbg_e1b[:], in_=d2[:])
                    nc.sync.dma_start(out=dbg_ppc[:], in_=ppc_of[0][:])
                drop(b)
            emit_mulred(BPC - 2)
            emit_mulred(BPC - 1)

    _split_excess_waits(nc)
    return nc


_NC_CACHE = None
LAST_RESULTS = None


def kernel(**inputs) -> np.ndarray:
    global _NC_CACHE
    x = np.asarray(inputs["x"], np.float32)
    edge_index = np.asarray(inputs["edge_index"])
    consts = _build_consts(
        edge_index,
        np.asarray(inputs["Wq"], np.float32), np.asarray(inputs["bq"], np.float32),
        np.asarray(inputs["Wk"], np.float32), np.asarray(inputs["bk"], np.float32),
        np.asarray(inputs["Wv"], np.float32), np.asarray(inputs["bv"], np.float32),
        np.asarray(inputs["Wo"], np.float32), np.asarray(inputs["bo"], np.float32),
    )

    if _NC_CACHE is None:
        _NC_CACHE = _build_program()
    nc = _NC_CACHE

    in_maps = []
    for core in range(NCORES):
        xs = x[core * BPC : (core + 1) * BPC]  # [BPC, N, F]
        xt = np.ascontiguousarray(xs.transpose(0, 2, 1))  # [BPC, F, N]
        m = {"xt": xt}
        m.update(consts)
        cb = consts["cblob"].copy()
        cb[:, OFF_XT0 : OFF_XT0 + 256] = xt[0].reshape(128, 256)
        m["cblob"] = np.ascontiguousarray(cb)
        in_maps.append(m)

    res = run_bass_kernel_spmd(nc, in_maps, list(range(NCORES)))
    global LAST_RESULTS
    LAST_RESULTS = res
    outs = [res.results[i]["out"] for i in range(NCORES)]
    return np.concatenate(outs, axis=0).astype(np.float32)


if __name__ == "__main__":
    rng = np.random.default_rng(0)
    demo = dict(
        x=rng.standard_normal((B, N, F), dtype=np.float32),
        edge_index=np.concatenate(
            [rng.integers(0, B, (2, 8192)), np.stack([np.arange(B)] * 2)], axis=1
        ).astype(np.int32),
        Wq=rng.standard_normal((F, H * D), dtype=np.float32) / np.sqrt(F),
        bq=rng.standard_normal(H * D, dtype=np.float32) / np.sqrt(F),
        Wk=rng.standard_normal((F, H * D), dtype=np.float32) / np.sqrt(F),
        bk=rng.standard_normal(H * D, dtype=np.float32) / np.sqrt(F),
        Wv=rng.standard_normal((F, H * D), dtype=np.float32) / np.sqrt(F),
        bv=rng.standard_normal(H * D, dtype=np.float32) / np.sqrt(F),
        Wo=rng.standard_normal((F, D)[0:1] and (H * D, D), dtype=np.float32) / np.sqrt(H * D),
        bo=rng.standard_normal(D, dtype=np.float32) / np.sqrt(H * D),
    )
    out = kernel(**demo)
    print("kernel output", out.shape, out.dtype)
